# revision 3
# baseline (speedup 1.0000x reference)
"""Conv2d 3x3 (stride 1, pad 1) as 9 shifted matmuls on TRN2, data-parallel
over batch across 8 NeuronCores.

Full shapes: img [32,128,112,112] f32, weight [256,128,3,3] f32, bias [256] f32
-> out [32,256,112,112] f32.

Per core: 4 images. C_in=128 is the contraction/partition dim. The image
lives in SBUF as a zero-padded [128, 114, 114] fp32r buffer; each PSUM tile
covers 4 output rows [128, 4, 112] and accumulates 9 matmuls (one per filter
tap) reading 3D-strided slices of the padded buffer. Weights are
host-transposed to [C_in, 9, 2, 128] so lhsT tiles are direct slices.
"""

import os
import sys

sys.path.insert(0, "/opt/trn_rl_repo")

import numpy as np

N_CORES = 8
N, C_IN, H, W = 32, 128, 112, 112
C_OUT, KH, KW = 256, 3, 3
PER_CORE = N // N_CORES           # 4 images
HP, WP = H + 2, W + 2             # padded 114 x 114
RPC = int(os.environ.get("CONV_RPC", "4"))  # output rows per PSUM tile
NCHUNK = H // RPC                 # 28 chunks
MT = C_OUT // 128                 # 2 C_out tiles

# matmul input dtype: "f32r" (fp32 relaxed, 1 cyc/row at N>=256), "bf16",
# or "f16" (same speed as bf16, 10-bit mantissa)
MM_DTYPE = os.environ.get("CONV_MM_DTYPE", "f16")
# tap-major loop order (groups of 8 chunks share a weight tile)
TAP_GROUPED = os.environ.get("CONV_TAP_GROUPED", "0") == "1"
# group size for tap-grouped mode (chunks sharing one LDWEIGHTS)
TAP_G = int(os.environ.get("CONV_TAP_G", "8"))
# alternate PSUM drains between VectorE and ScalarE
DRAIN_SPLIT = os.environ.get("CONV_DRAIN_SPLIT", "0") == "1"
# diagnostics (timing-only, wrong math): "samewt" uses tap0/mt0 weights for
# every matmul so consecutive matmuls share an identical weights AP
DIAG = os.environ.get("CONV_DIAG", "")
# "3d" (baseline rhs [128,4,112] row slices) or "flat456" (rhs is a single
# contiguous [128,456] window spanning 4 padded rows; the 2 seam columns per
# row land in discarded PSUM columns)
LAYOUT = os.environ.get("CONV_LAYOUT", "3d")

_CACHED = {}

# strip tile-inserted InstLdweights + enable the walrus LDW optimization
# (self-loading matmuls; walrus backgrounds/dedupes the weight loads)
STRIP_LDW = os.environ.get("STRIP_LDW", "0") == "1"
# drop an InstLdweights when the immediately preceding LDW in the same block
# loaded the identical weights AP (load-once-stream-many); pairs with
# CONV_TAP_GROUPED=1 so the scheduler emits runs of same-weight matmuls
DEDUP_LDW = os.environ.get("DEDUP_LDW", "0") == "1"


def _enable_walrus_ldw_opt():
    """Flip --enable-ldw-opt for our NEFF compile. Only valid when the BIR
    carries no explicit InstLdweights (walrus rejects that combination)."""
    import concourse.bass_utils as bu
    if getattr(bu.run_command, "_ldw_opt_wrapped", False):
        return
    orig = bu.run_command

    def _run(argv, **kw):
        argv = ["--enable-ldw-opt=true" if a == "--enable-ldw-opt=false"
                else a for a in argv]
        return orig(argv, **kw)

    _run._ldw_opt_wrapped = True
    bu.run_command = _run


def _build(repeat: int = 1):
    import contextlib
    import concourse.tile as tile
    import concourse.mybir as mybir
    from concourse import bacc

    F32 = mybir.dt.float32
    CDT = {"f32r": mybir.dt.float32r, "bf16": mybir.dt.bfloat16,
           "f16": mybir.dt.float16, "f8e4": mybir.dt.float8e4,
           "f8e3": mybir.dt.float8e3}[MM_DTYPE]

    nc = bacc.Bacc("TRN2", target_bir_lowering=False, debug=False)
    img_d = nc.dram_tensor("img", [PER_CORE, C_IN, H, W], F32,
                           kind="ExternalInput").ap()
    wt_d = nc.dram_tensor("wt", [C_IN, KH * KW, MT, 128], F32,
                          kind="ExternalInput").ap()
    bias_d = nc.dram_tensor("bias", [128, MT], F32, kind="ExternalInput").ap()
    out_d = nc.dram_tensor("out", [PER_CORE, C_OUT, H, W], F32,
                           kind="ExternalOutput").ap()

    with tile.TileContext(nc) as tc:
        with tc.tile_pool(name="const", bufs=1) as const_pool, \
             tc.tile_pool(name="imgpad", bufs=2) as imgpad_pool, \
             tc.tile_pool(name="ldchunk", bufs=4) as ld_pool, \
             tc.tile_pool(name="outsb", bufs=4) as out_pool, \
             tc.tile_pool(name="psum", bufs=8, space="PSUM") as psum_pool:

            # ---- constants: weights (converted to CDT) + bias ----
            wt_f32 = const_pool.tile([C_IN, KH * KW, MT, 128], F32)
            nc.sync.dma_start(wt_f32[:], wt_d[:])
            wt_c = const_pool.tile([C_IN, KH * KW, MT, 128], CDT)
            nc.vector.tensor_copy(out=wt_c[:], in_=wt_f32[:])
            bias_sb = const_pool.tile([128, MT], F32)
            nc.sync.dma_start(bias_sb[:], bias_d[:])
            zrow = const_pool.tile([C_IN, WP], F32)
            nc.vector.memset(zrow[:], 0.0)

            # repeat>1 wraps the whole body in a hardware loop for timing
            # amplification (identical work each iteration, same output).
            loop_ctx = tc.For_i(0, repeat, 1) if repeat > 1 \
                else contextlib.nullcontext()
            with loop_ctx:
              for i in range(PER_CORE):
                # ---- load + convert image i into padded CDT buffer ----
                if LAYOUT == "flat456":
                    # flat buffer with 8 elements of slack so the last
                    # chunk's shifted windows stay in-bounds
                    imgpf = imgpad_pool.tile([C_IN, HP * WP + 8], CDT)
                    imgp = imgpf[:, 0:HP * WP].rearrange(
                        "p (h w) -> p h w", h=HP)
                    nc.vector.tensor_copy(out=imgpf[:, HP * WP:],
                                          in_=zrow[:, 0:8])
                else:
                    imgpf = None
                    imgp_t = imgpad_pool.tile([C_IN, HP, WP], CDT)
                    imgp = imgp_t[:]
                # fp32r tiles cannot be memset; zero the pads via DVE copy
                # from an f32 zero row (a legal fp32r-rounding producer).
                nc.vector.tensor_copy(out=imgp[:, 0, :], in_=zrow[:])
                nc.vector.tensor_copy(out=imgp[:, HP - 1, :], in_=zrow[:])
                nc.vector.tensor_copy(out=imgp[:, 0:HP - 1, WP - 1],
                                      in_=zrow[:, 0:HP - 1])
                nc.vector.tensor_copy(out=imgp[:, 1:HP, 0],
                                      in_=zrow[:, 0:HP - 1])
                for c in range(NCHUNK):
                    ld = ld_pool.tile([C_IN, RPC, W], F32)
                    nc.sync.dma_start(ld[:], img_d[i, :, c * RPC:(c + 1) * RPC, :])
                    nc.vector.tensor_copy(
                        out=imgp[:, 1 + c * RPC: 1 + (c + 1) * RPC, 1:1 + W],
                        in_=ld[:])

                # ---- conv: 2 C_out tiles x 28 row-chunks x 9 taps ----
                def wsel(t, mt):
                    if DIAG == "samewt":
                        return wt_c[:, 0, 0, :]
                    return wt_c[:, t, mt, :]

                def rsel(c, ky, kx):
                    if DIAG == "aligned":
                        kx = 0
                    if LAYOUT == "flat456":
                        base = (c * RPC + ky) * WP + kx
                        return imgpf[:, base: base + RPC * WP]
                    return imgp[:, c * RPC + ky: c * RPC + ky + RPC, kx: kx + W]

                def drain(ps_in, c, mt, j):
                    osb = out_pool.tile([128, RPC, W], F32)
                    if DRAIN_SPLIT and j % 2 == 1:
                        nc.scalar.activation(
                            osb[:], ps_in,
                            func=mybir.ActivationFunctionType.Copy,
                            bias=bias_sb[:, mt:mt + 1])
                    else:
                        nc.vector.tensor_scalar_add(osb[:], ps_in,
                                                    bias_sb[:, mt:mt + 1])
                    nc.sync.dma_start(
                        out_d[i, mt * 128:(mt + 1) * 128,
                              c * RPC:(c + 1) * RPC, :],
                        osb[:])

                PSW = WP if LAYOUT == "flat456" else W
                for mt in range(MT):
                    if not TAP_GROUPED:
                        # chunk-major: per chunk, 9 rotating-weight matmuls
                        for c in range(NCHUNK):
                            ps = psum_pool.tile([128, RPC, PSW], F32)
                            for t in range(KH * KW):
                                ky, kx = divmod(t, KW)
                                nc.tensor.matmul(
                                    ps[:],
                                    lhsT=wsel(t, mt),
                                    rhs=rsel(c, ky, kx),
                                    start=(t == 0), stop=(t == KH * KW - 1),
                                )
                            if DIAG != "nodrain":
                                drain(ps[:, :, 0:W], c, mt, c)
                    else:
                        # tap-major over groups of G chunks: G consecutive
                        # matmuls share one weight tile (amortized LDW)
                        G = TAP_G
                        for c0 in range(0, NCHUNK, G):
                            cs = list(range(c0, min(c0 + G, NCHUNK)))
                            pss = []
                            for _ in cs:
                                ps_g = psum_pool.tile([128, RPC, W], F32,
                                                      tag="ps")
                                pss.append(ps_g)
                            for t in range(KH * KW):
                                ky, kx = divmod(t, KW)
                                for ps, c in zip(pss, cs):
                                    nc.tensor.matmul(
                                        ps[:],
                                        lhsT=wsel(t, mt),
                                        rhs=imgp[:, c * RPC + ky:
                                                 c * RPC + ky + RPC,
                                                 kx: kx + W],
                                        start=(t == 0),
                                        stop=(t == KH * KW - 1),
                                    )
                            for j, (ps, c) in enumerate(zip(pss, cs)):
                                drain(ps[:], c, mt, j)

    if STRIP_LDW:
        _strip_ldweights(nc, mybir)
        _enable_walrus_ldw_opt()
    if DEDUP_LDW:
        _dedup_ldweights(nc, mybir)
    nc.compile()
    return nc


def _dedup_ldweights(nc, mybir):
    """Delete an InstLdweights whose weights AP is byte-identical to the
    previous InstLdweights in the same block (the PE array still holds those
    weights; only matmuls/ldweights touch it). Waits on a deleted LDW move to
    the next instruction. Tracking resets at block boundaries so hardware
    loops stay conservative."""
    removed = 0
    for f in nc.m.functions:
        for blk in f.blocks:
            ins = blk.instructions
            last_sig = None
            i = 0
            while i < len(ins):
                inst = ins[i]
                nm = type(inst).__name__
                if nm == "InstLdweights":
                    c = inst.ins[0].concise
                    sig = c() if callable(c) else c
                    if sig == last_sig:
                        si = inst.sync_info
                        waits = list(si.on_wait) if si is not None else []
                        ups = list(si.on_update) if si is not None else []
                        if (waits or ups) and i + 1 < len(ins):
                            nxt = ins[i + 1]
                            nsi = nxt.sync_info
                            ow = list(nsi.on_wait) if nsi else []
                            ou = list(nsi.on_update) if nsi else []
                            nxt.sync_info = mybir.SyncInfo(
                                on_wait=ow + waits, on_update=ou + ups)
                        del ins[i]
                        removed += 1
                        continue
                    last_sig = sig
                i += 1
    print(f"dedup_ldweights: removed {removed} InstLdweights")


def _strip_ldweights(nc, mybir):
    """Remove tile-inserted InstLdweights (matmuls are self-loading; walrus
    regenerates weight loads internally, and its LDW optimization pass only
    runs when no explicit InstLdweights is present). Waits attached to a
    stripped Ldweights are re-merged onto the following matmul."""
    for f in nc.m.functions:
        for blk in f.blocks:
            ins = blk.instructions
            i = 0
            while i < len(ins):
                inst = ins[i]
                if isinstance(inst, mybir.InstLdweights):
                    si = inst.sync_info
                    waits = list(si.on_wait) if si is not None else []
                    if waits and i + 1 < len(ins):
                        nxt = ins[i + 1]
                        nsi = nxt.sync_info
                        if nsi is None:
                            nxt.sync_info = mybir.SyncInfo(
                                on_wait=waits, on_update=[])
                        else:
                            nxt.sync_info = mybir.SyncInfo(
                                on_wait=list(nsi.on_wait) + waits,
                                on_update=list(nsi.on_update))
                    del ins[i]
                else:
                    i += 1


def _make_runner(nc, donate=True):
    """Build a cached sharded-jit runner for `nc` on 8 cores.

    Mirrors bass2jax.run_bass_via_pjrt's multi-core path, but keeps the
    jitted function so repeated calls reuse the compiled executable (the
    stock helper rebuilds the jit -> reruns the minutes-long NEFF compile
    every call). With donate=False, inputs (incl. the zero output seeds)
    can live on device and be reused across timing reps.
    """
    import jax
    import jax.numpy as jnp
    from jax.sharding import Mesh, PartitionSpec, NamedSharding
    from jax.experimental.shard_map import shard_map
    import concourse.mybir as mybir
    from concourse import bass2jax

    bass2jax.install_neuronx_cc_hook()

    partition_name = nc.partition_id_tensor.name if nc.partition_id_tensor else None
    in_names, out_names, out_avals, zero_outs = [], [], [], []
    for alloc in nc.m.functions[0].allocations:
        if not isinstance(alloc, mybir.MemoryLocationSet):
            continue
        name = alloc.memorylocations[0].name
        if alloc.kind == "ExternalInput":
            if name != partition_name:
                in_names.append(name)
        elif alloc.kind == "ExternalOutput":
            shape = tuple(alloc.tensor_shape)
            dtype = mybir.dt.np(alloc.dtype)
            out_names.append(name)
            out_avals.append(jax.core.ShapedArray(shape, dtype))
            zero_outs.append(np.zeros(shape, dtype))
    n_params = len(in_names)
    n_outs = len(out_avals)
    all_in_names = list(in_names) + list(out_names)
    if partition_name is not None:
        all_in_names.append(partition_name)

    def _body(*args):
        operands = list(args)
        if partition_name is not None:
            operands.append(bass2jax.partition_id_tensor())
        outs = bass2jax._bass_exec_p.bind(
            *operands,
            out_avals=tuple(out_avals),
            in_names=tuple(all_in_names),
            out_names=tuple(out_names),
            lowering_input_output_aliases=(),
            sim_require_finite=True,
            sim_require_nnan=True,
            nc=nc,
        )
        return tuple(outs)

    devices = jax.devices()[:N_CORES]
    mesh = Mesh(np.asarray(devices), ("core",))
    in_specs = (PartitionSpec("core"),) * (n_params + n_outs)
    out_specs = (PartitionSpec("core"),) * len(out_names)
    kwargs = dict(keep_unused=True)
    if donate:
        kwargs["donate_argnums"] = tuple(range(n_params, n_params + n_outs))
    sharded = jax.jit(
        shard_map(_body, mesh=mesh, in_specs=in_specs, out_specs=out_specs,
                  check_rep=False),
        **kwargs)
    sharding = NamedSharding(mesh, PartitionSpec("core"))

    def prep(in_maps, device_put=False):
        """concat per-core inputs (+ zero output seeds) to global arrays."""
        concat = [np.concatenate([np.asarray(m[name]) for m in in_maps], axis=0)
                  for name in in_names]
        concat += [np.concatenate([z] * N_CORES, axis=0) for z in zero_outs]
        if device_put:
            import jax
            concat = [jax.device_put(a, sharding) for a in concat]
        return concat

    def run(args):
        outs = sharded(*args)
        return outs

    def to_results(outs):
        results = [dict() for _ in range(N_CORES)]
        for name, arr in zip(out_names, outs):
            arr = np.asarray(arr)
            per = np.split(arr, N_CORES, axis=0)
            for c in range(N_CORES):
                results[c][name] = per[c]
        return results

    return prep, run, to_results


def kernel(img: np.ndarray, weight: np.ndarray, bias: np.ndarray) -> np.ndarray:
    img = np.ascontiguousarray(np.asarray(img, dtype=np.float32))
    weight = np.ascontiguousarray(np.asarray(weight, dtype=np.float32))
    bias = np.ascontiguousarray(np.asarray(bias, dtype=np.float32))

    # host-side weight/bias rearrangement (tiny): lhsT layout [C_in, tap, mt, 128]
    wt = np.ascontiguousarray(
        weight.transpose(1, 2, 3, 0).reshape(C_IN, KH * KW, MT, 128))
    bias2 = np.ascontiguousarray(bias.reshape(MT, 128).T)

    if "nc" not in _CACHED:
        _CACHED["nc"] = _build()
        _CACHED["runner"] = _make_runner(_CACHED["nc"], donate=False)
    prep, run, to_results = _CACHED["runner"]

    shards = img.reshape(N_CORES, PER_CORE, C_IN, H, W)
    in_maps = [{"img": shards[i], "wt": wt, "bias": bias2}
               for i in range(N_CORES)]

    outs = run(prep(in_maps))
    results = to_results(outs)
    _CACHED["last_results"] = results
    return np.concatenate([r["out"] for r in results], axis=0)

